# revision 34
# baseline (speedup 1.0000x reference)
"""Trainium2 Bass kernel for nn_EncoderLayer (GNN message passing, 2-relation GAT).

v3 design (pair-shared z-table):
  - Each trn2 chip pair (2p, 2p+1) shares HBM; the projection table
    ZT[10000, 896] bf16 (per rel) lives in pair-Shared DRAM. The even core
    computes+writes rows [0:5000), the odd core rows [5000:10000) (If-branch
    on core parity picks write addresses; compute uses per-core input data).
    A tiny pair AllGather is the phase-1 -> phase-2 barrier.
  - Table row: 12 x [z(t) 64 (k,h-order) | el(t) 4] | er 48 | pad. The
    (k,h) column order (host-permuted weights) makes the phase-2 ex*z
    broadcast multiply a packed-inner-dim DVE op (2x mode) with no
    scalar broadcast-exp. er for the window's own dst rows is gathered
    back from the table tail (256B suffix gather, -1 padded idx).
  - BN rsqrt = DVE reciprocal + scalar Sqrt; BN2 coefs batched between
    phases => only ~4 activation-table loads total (sqrt/exp/sqrt/gelu).
  - er expansion to edges on PE (S-block transposes, batched PSUM->SBUF
    copy, one-hot matmul with er rows).
  - Phase 2: gather z rows per edge, ex = exp(leaky(el+er)),
    msg = [ex*z | ex], segment-sum via S-matmuls in PSUM, alpha-normalize,
    residual, BN2 stats. Phase 3: BN2 apply + FFN (t-pair transposes,
    host-permuted ffw1 rows / ffw2 cols; final add restores layout).
"""

import sys

sys.path.insert(0, "/opt/trn_rl_repo")

import numpy as np
import ml_dtypes

import concourse.bass as bass
import concourse.bacc as bacc
import concourse.tile as tile
import concourse.mybir as mybir
from concourse.bass_utils import run_bass_kernel_spmd

F32 = mybir.dt.float32
BF16 = mybir.dt.bfloat16
I16 = mybir.dt.int16
AF = mybir.ActivationFunctionType
ALU = mybir.AluOpType
BF16NP = ml_dtypes.bfloat16

N, T, D, H, DH, DFF = 10000, 12, 64, 4, 16, 128
NCORES = 8
CHUNK = N // NCORES          # 1250
WIN = 128
NW = (CHUNK + WIN - 1) // WIN  # 10
HALF = N // 2                # 5000 rows per pair member
HBLK = (HALF + 127) // 128   # 40 blocks (last = 8 rows)
ZROW = 896                   # bf16 elems per table row (1792 B)
EPS = 1e-5
NEG_SLOPE = 0.2
_SIM = False                 # True: branch-free variant for TimelineSim
_DEBUG = False               # True: add ZDBG output dumping table rows


def _win_nodes(w):
    return min(WIN, CHUNK - w * WIN)


def _hblk_nodes(j):
    return min(128, HALF - j * 128)


def _idx16(arr):
    """dma_gather index layout: idx i at [i%16, i//16], tiled to 128 rows."""
    return np.ascontiguousarray(np.tile(arr.reshape(-1, 16).T, (8, 1)))


def _prep_core_rel(src, dst, lo, B):
    hi = lo + CHUNK
    sel = (dst >= lo) & (dst < hi)
    es = src[sel].astype(np.int64)
    ed = (dst[sel] - lo).astype(np.int64)
    order = np.argsort(ed, kind="stable")
    es, ed = es[order], ed[order]
    L = NW * B * 128
    src_arr = np.zeros(L, np.int16)
    S = np.zeros((NW, 128, B * 128), BF16NP)
    wstart = np.searchsorted(ed, np.arange(NW) * WIN)
    wend = np.searchsorted(ed, np.arange(1, NW + 1) * WIN)
    for w in range(NW):
        seg_src = es[wstart[w]:wend[w]]
        seg_dst = ed[wstart[w]:wend[w]]
        cnt = len(seg_src)
        assert cnt <= B * 128
        base = w * B * 128
        src_arr[base:base + cnt] = seg_src
        i = np.arange(cnt)
        S[w, i % 128, (i // 128) * 128 + (seg_dst - w * WIN)] = 1.0
    return _idx16(src_arr), S


def _max_blocks(src, dst):
    best = 0
    for m in range(NCORES):
        lo = m * CHUNK
        sel = (dst >= lo) & (dst < lo + CHUNK)
        ed = dst[sel] - lo
        cnt = np.bincount(ed // WIN, minlength=NW)
        best = max(best, int(np.max((cnt + 127) // 128)))
    return best


def _build_program(B):
    nc = bacc.Bacc("TRN2", target_bir_lowering=False, debug=False,
                   num_devices=NCORES)
    BL = B * 128
    L16W = BL // 16

    # ---- DRAM tensors ----
    xh_in = nc.dram_tensor("xh", [HBLK * 128, T * D], BF16, kind="ExternalInput")
    bn1r_in = nc.dram_tensor("bn1r", [128, HBLK * 2], F32, kind="ExternalInput")
    xc_in = nc.dram_tensor("xc", [CHUNK, T * D], F32, kind="ExternalInput")
    bn2c_in = nc.dram_tensor("bn2c", [CHUNK, 2], F32, kind="ExternalInput")
    wx_in, s_in, si_in = [], [], []
    for r in (1, 2):
        wx_in.append(nc.dram_tensor(f"Wx{r}", [D, 72], F32, kind="ExternalInput"))
        s_in.append(nc.dram_tensor(f"S{r}", [NW, 128, BL], BF16,
                                   kind="ExternalInput"))
        si_in.append(nc.dram_tensor(f"srcidx{r}", [128, NW * L16W], I16,
                                    kind="ExternalInput"))
    ni_in = nc.dram_tensor("nodeidx", [128, NW * 8], I16, kind="ExternalInput")
    ffw1_in = nc.dram_tensor("ffw1", [D, DFF], F32, kind="ExternalInput")
    ffb1_in = nc.dram_tensor("ffb1", [DFF, 1], F32, kind="ExternalInput")
    ffw2_in = nc.dram_tensor("ffw2", [DFF, D], F32, kind="ExternalInput")
    ffb2_in = nc.dram_tensor("ffb2", [D, 1], F32, kind="ExternalInput")
    ident_in = nc.dram_tensor("ident", [128, 128], BF16, kind="ExternalInput")
    out_d = nc.dram_tensor("OUT", [CHUNK, T * D], F32, kind="ExternalOutput")
    if _DEBUG:
        zdbg_d = nc.dram_tensor("ZDBG", [2, 128, 1792], BF16,
                                kind="ExternalOutput")

    ZTall = nc.dram_tensor("ZT", [N, 2 * ZROW], BF16, kind="Internal",
                           addr_space="Shared")


    with tile.TileContext(nc) as tc:
        with tc.tile_pool(name="const", bufs=1) as cpool:
            # ---- Phase 0 ----
            ident = cpool.tile([128, 128], BF16)
            nc.sync.dma_start(ident[:], ident_in[:])
            wd2 = cpool.tile([128, 2, 2, 72], BF16, tag="wd2")
            nc.vector.memset(wd2[:], 0.0)
            for r in range(2):
                wf = cpool.tile([D, 72], F32, tag="wf", bufs=2)
                nc.sync.dma_start(wf[:], wx_in[r][:])
                for par in range(2):
                    nc.vector.tensor_copy(
                        wd2[par * 64:par * 64 + 64, r, par, :], wf[:])
            ffw1 = cpool.tile([128, DFF], BF16)
            t1 = cpool.tile([D, DFF], F32, tag="t1")
            nc.sync.dma_start(t1[:], ffw1_in[:])
            nc.vector.tensor_copy(ffw1[0:D, :], t1[:])
            nc.sync.dma_start(ffw1[64:128, :], ffw1[0:64, :])
            ffw2 = cpool.tile([DFF, D], BF16)
            t2 = cpool.tile([DFF, D], F32, tag="t2")
            nc.sync.dma_start(t2[:], ffw2_in[:])
            nc.vector.tensor_copy(ffw2[:], t2[:])
            ffb1 = cpool.tile([DFF, 1], F32)
            nc.sync.dma_start(ffb1[:], ffb1_in[:])
            ffb2r = cpool.tile([128, 1], F32)
            nc.sync.dma_start(ffb2r[0:64, :], ffb2_in[:])
            nc.sync.dma_start(ffb2r[64:128, :], ffb2_in[:])
            si_sb = []
            for r in range(2):
                sit = cpool.tile([128, NW * L16W], I16, tag=f"si{r}")
                nc.sync.dma_start(sit[:], si_in[r][:])
                si_sb.append(sit)
            ni_sb = cpool.tile([128, NW * 8], I16, tag="ni")
            nc.sync.dma_start(ni_sb[:], ni_in[:])
            gb1 = cpool.tile([128, HBLK, 2], F32, tag="gb1")
            nc.sync.dma_start(gb1[:].rearrange("p a b -> p (a b)"), bn1r_in[:])

            if not _SIM:
                par_reg = nc.sync.alloc_register("par")
                nc.sync.reg_mod(par_reg, nc.sync.partition_id(), 2)
                par_sv = nc.sync.snap(par_reg, min_val=0, max_val=1)

            # ---- Phase 1: 40 half-table blocks ----
            with (
                tc.tile_pool(name="p1", bufs=4) as p1,
                tc.tile_pool(name="p1s", bufs=3) as p1s,
                tc.tile_pool(name="p1t", bufs=2, space="PSUM") as p1t,
                tc.tile_pool(name="p1z", bufs=2, space="PSUM") as p1z,
                tc.tile_pool(name="zelp", bufs=4) as zelp,
            ):
                xt4 = None
                for j in range(HBLK):
                    n0 = j * 128
                    nb = _hblk_nodes(j)
                    if j % 4 == 0:
                        xt4 = p1.tile([128, 4, T * D], BF16, tag="xt")
                        nc.sync.dma_start(
                            xt4[:],
                            xh_in[j * 128:(j + 4) * 128].rearrange(
                                "(q p) c -> p q c", p=128))
                    xt = xt4[:, j % 4, :]
                    st6 = p1s.tile([128, 2, 6], F32, tag="st6")
                    nc.vector.bn_stats(st6[:nb, 0, :], xt[:nb, 0:384])
                    nc.vector.bn_stats(st6[:nb, 1, :], xt[:nb, 384:768])
                    mv = p1s.tile([128, 2], F32, tag="mv")
                    nc.vector.bn_aggr(mv[:nb], st6[:nb])
                    gb = gb1[:, j, :]
                    # rsqrt = sqrt(1/(v+eps)): no exp/ln tables needed
                    rs = p1s.tile([128, 1], F32, tag="rs")
                    nc.vector.tensor_scalar_add(rs[:nb], mv[:nb, 1:2], EPS)
                    nc.vector.reciprocal(rs[:nb], rs[:nb])
                    nc.scalar.activation(rs[:nb], rs[:nb], AF.Sqrt)
                    a = p1s.tile([128, 1], F32, tag="a")
                    nc.vector.tensor_mul(a[:nb], gb[:nb, 0:1], rs[:nb])
                    b = p1s.tile([128, 1], F32, tag="b")
                    nc.vector.tensor_mul(b[:nb], a[:nb], mv[:nb, 0:1])
                    nc.vector.tensor_sub(b[:nb], gb[:nb, 1:2], b[:nb])
                    h = p1.tile([128, T * D], BF16, tag="h")
                    nc.scalar.activation(h[:nb], xt[:nb], AF.Identity,
                                         bias=b[:nb], scale=a[:nb])
                    tp = p1t.tile([128, 6, 128], BF16, tag="tp")
                    for t2 in range(6):
                        nc.tensor.transpose(tp[:, t2, 0:nb],
                                            h[:nb, t2 * 128:(t2 + 1) * 128],
                                            ident[:nb, :nb])
                    ht = p1.tile([128, 6, 128], BF16, tag="ht")
                    nc.scalar.activation(ht[:, :, 0:nb], tp[:, :, 0:nb], AF.Copy)
                    zel = zelp.tile([128, 2, ZROW], BF16, tag="zel")
                    for r in range(2):
                        zp = p1z.tile([128, 6, 256], F32, tag="zp")
                        zp4 = zp[:, :, 0:144].rearrange(
                            "p a (q c) -> p a q c", c=72)
                        for t2 in range(6):
                            nc.tensor.matmul(
                                zp[:nb, t2, 0:144],
                                ht[:, t2, 0:nb],
                                wd2[:, r, :, :].rearrange("p a b -> p (a b)"),
                                start=True, stop=True)
                        for par in range(2):
                            # z+el fused: 68 cols per t, (k,h) weight order
                            outz = zel[0:nb, r, 0:816] \
                                .rearrange("p (a q c) -> p a q c",
                                           q=2, c=68)[:, :, par, :]
                            inz = zp4[0:nb, :, par, 0:68]
                            if par == 0:
                                nc.scalar.activation(outz, inz, AF.Copy)
                            else:
                                nc.vector.tensor_copy(outz, inz)
                        # er tail: cols 816:864, order (t, h)
                        oute = zel[0:nb, r, 816:864] \
                            .rearrange("p (a q c) -> p a q c", q=2, c=4)
                        ine = zp4[0:nb, :, :, 68:72]
                        nc.vector.tensor_copy(oute, ine)
                    if _SIM:
                        nc.sync.dma_start(ZTall[n0:n0 + nb, :],
                                          zel[0:nb, :, :])
                    else:
                        with tc.If(par_sv == 0):
                            nc.sync.dma_start(ZTall[n0:n0 + nb, :],
                                              zel[0:nb, :, :])
                        with tc.If(par_sv == 1):
                            nc.sync.dma_start(
                                ZTall[HALF + n0:HALF + n0 + nb, :],
                                zel[0:nb, :, :])

            tc.strict_bb_all_engine_barrier()

            # ---- pair barrier ----
            with tc.tile_pool(name="ccd", bufs=1, space="DRAM") as ccd:
                cbin = ccd.tile([1, 32], F32, tag="cbin")
                cbout = ccd.tile([8, 32], F32, tag="cbout")
                bar_sb = cpool.tile([1, 32], F32, tag="barsb")
                nc.vector.memset(bar_sb[:], 0.0)
                nc.gpsimd.dma_start(cbin[:], bar_sb[:])
                nc.gpsimd.collective_compute(
                    "AllGather", ALU.bypass,
                    replica_groups=[[0, 1, 2, 3, 4, 5, 6, 7]],
                    ins=[cbin[:].opt()], outs=[cbout[:].opt()],
                )
                nc.gpsimd.dma_start(bar_sb[:], cbout[0:1, :])
            tc.strict_bb_all_engine_barrier()

            if _DEBUG:
                zd = cpool.tile([128, 2, 1792], BF16, tag="zd")
                nc.gpsimd.dma_start(zd[:, 0, :], ZTall[0:128, :])
                nc.gpsimd.dma_start(zd[:, 1, :], ZTall[HALF:HALF + 128, :])
                nc.gpsimd.dma_start(zdbg_d[0], zd[:, 0, :])
                nc.gpsimd.dma_start(zdbg_d[1], zd[:, 1, :])

            # ---- Phase 2 ----
            x2_tiles = []
            mv2_tiles = []
            with (
                tc.tile_pool(name="x2p", bufs=NW) as x2p,
                tc.tile_pool(name="mv2p", bufs=NW) as mv2p,
                tc.tile_pool(name="abp", bufs=1) as abp,
            ):
                with (
                    tc.tile_pool(name="zg", bufs=2) as zgp,
                    tc.tile_pool(name="sp", bufs=2) as spp,
                    tc.tile_pool(name="erw", bufs=2) as erwp,
                    tc.tile_pool(name="lkp", bufs=2) as lkp,
                    tc.tile_pool(name="msg", bufs=6) as msgp,
                    tc.tile_pool(name="p2s", bufs=4) as p2s,
                    tc.tile_pool(name="p2t", bufs=2) as p2t,
                    tc.tile_pool(name="stsp", bufs=2) as stsp,
                    tc.tile_pool(name="msum", bufs=1, space="PSUM") as msump,
                    tc.tile_pool(name="erp", bufs=1, space="PSUM") as erpp,
                    tc.tile_pool(name="stp", bufs=1, space="PSUM") as stpp,
                ):
                    for w in range(NW):
                        nw = _win_nodes(w)
                        ms = msump.tile([128, 2, 1024], F32, tag="msum")
                        for r in range(2):
                            ssb = spp.tile([128, BL], BF16, tag=f"ssb{r}")
                            nc.sync.dma_start(ssb[:], s_in[r][w])
                            zg = zgp.tile([128, B, ZROW], BF16, tag=f"zg{r}")
                            nc.gpsimd.dma_gather(
                                zg[:], ZTall[:, r * ZROW:r * ZROW + ZROW],
                                si_sb[r][:, w * L16W:(w + 1) * L16W],
                                BL, BL, ZROW, elem_step=2 * ZROW,
                                single_packet=False)
                            erw = erwp.tile([128, 128], BF16, tag=f"erw{r}")
                            nc.vector.memset(erw[:], 0.0)
                            nc.gpsimd.dma_gather(
                                erw[:].unsqueeze(1),
                                ZTall[:, r * ZROW + 768:r * ZROW + 896],
                                ni_sb[:, w * 8:(w + 1) * 8],
                                128, 128, 128, elem_step=2 * ZROW,
                                single_packet=False)
                            # er expansion: St_b = S_b^T (batched), one-hot mm
                            stp = stpp.tile([128, B, 128], BF16, tag="stp")
                            for b in range(B):
                                nc.tensor.transpose(
                                    stp[:, b, :],
                                    ssb[:, b * 128:(b + 1) * 128], ident[:])
                            sts = stsp.tile([128, B, 128], BF16, tag="sts")
                            nc.vector.tensor_copy(sts[:], stp[:])
                            er_ps = erpp.tile([128, B, 64], F32, tag="erps")
                            for b in range(B):
                                nc.tensor.matmul(
                                    er_ps[:, b, 0:T * H], sts[:, b, :],
                                    erw[:, 48:96], start=True, stop=True)
                            lk = lkp.tile([128, B, T * H], BF16, tag="lk")
                            nc.vector.tensor_add(
                                lk[:].rearrange("p b (t h) -> p b t h", h=H),
                                zg[:, :, 0:816].rearrange(
                                    "p b (t c) -> p b t c", c=68)[:, :, :, 64:68],
                                er_ps[:, :, 0:T * H].rearrange(
                                    "p b (t h) -> p b t h", h=H))
                            nc.vector.scalar_tensor_tensor(
                                lk[:], lk[:], NEG_SLOPE, lk[:], ALU.mult, ALU.max)
                            for b in range(B):
                                msgb = msgp.tile([128, 816], BF16, tag="msg")
                                nc.scalar.activation(msgb[:, 768:816],
                                                     lk[:, b, :], AF.Exp)
                                zv = zg[:, b, 0:816].rearrange(
                                    "p (t c) -> p t c", c=68)[:, :, 0:64] \
                                    .rearrange("p t (k h) -> p t k h", h=4)
                                nc.vector.tensor_mul(
                                    msgb[:, 0:768].rearrange(
                                        "p (t k h) -> p t k h", k=16, h=4),
                                    zv,
                                    msgb[:, 768:816].rearrange(
                                        "p (t h) -> p t h", h=4).unsqueeze(2)
                                    .broadcast_to((128, T, 16, H)))
                                lhsT = ssb[:, b * 128:(b + 1) * 128]
                                nc.tensor.matmul(ms[:, r, 0:512], lhsT,
                                                 msgb[:, 0:512],
                                                 start=(b == 0),
                                                 stop=(b == B - 1))
                                nc.tensor.matmul(ms[:, r, 512:816], lhsT,
                                                 msgb[:, 512:816],
                                                 start=(b == 0),
                                                 stop=(b == B - 1))
                        # epilogue
                        xcw = p2t.tile([128, T * D], F32, tag="xcw")
                        nc.sync.dma_start(xcw[:nw], xc_in[w * WIN:w * WIN + nw])
                        x2w = x2p.tile([128, T * D], F32, tag="x2")
                        mtmp = p2t.tile([128, T * D], F32, tag="mtmp")
                        for r in range(2):
                            rec = p2s.tile([128, T * H], F32, tag="rec")
                            nc.vector.tensor_scalar_max(
                                rec[:nw], ms[:nw, r, 768:816], 1e-16)
                            nc.vector.reciprocal(rec[:nw], rec[:nw])
                            rb = rec[:nw].rearrange(
                                "p (t h) -> p t h", h=H).unsqueeze(2) \
                                .broadcast_to((nw, T, 16, H))
                            dst = (x2w if r == 0 else mtmp)
                            nc.vector.tensor_mul(
                                dst[:nw].rearrange(
                                    "p (t k h) -> p t k h", k=16, h=4),
                                ms[:nw, r, 0:768].rearrange(
                                    "p (t k h) -> p t k h", k=16, h=4), rb)
                        # x2 kept in (t,k,h) order; xc viewed permuted
                        nc.vector.tensor_add(x2w[:nw], x2w[:nw], mtmp[:nw])
                        nc.vector.tensor_add(
                            x2w[:nw].rearrange("p (t k h) -> p t k h",
                                               k=16, h=4),
                            x2w[:nw].rearrange("p (t k h) -> p t k h",
                                               k=16, h=4),
                            xcw[:nw].rearrange("p (t h k) -> p t k h",
                                               h=4, k=16))
                        x2_tiles.append(x2w)
                        st6 = p2s.tile([128, 2, 6], F32, tag="st6b")
                        nc.vector.bn_stats(st6[:nw, 0, :], x2w[:nw, 0:384])
                        nc.vector.bn_stats(st6[:nw, 1, :], x2w[:nw, 384:768])
                        mv2 = mv2p.tile([128, 2], F32, tag="mvb")
                        nc.vector.bn_aggr(mv2[:nw], st6[:nw])
                        mv2_tiles.append(mv2)

                # ---- batched BN2 coefs (one Sqrt op: no table thrash) ----
                gb2 = abp.tile([128, NW, 2], F32, tag="gb2")
                for w in range(NW):
                    nw = _win_nodes(w)
                    nc.sync.dma_start(gb2[:nw, w, :],
                                      bn2c_in[w * WIN:w * WIN + nw])
                rs2 = abp.tile([128, NW], F32, tag="rs2")
                for w in range(NW):
                    nc.vector.tensor_scalar_add(rs2[:, w:w + 1],
                                                mv2_tiles[w][:, 1:2], EPS)
                nc.vector.reciprocal(rs2[:], rs2[:])
                nc.scalar.activation(rs2[:], rs2[:], AF.Sqrt)
                ab2 = abp.tile([128, NW, 2], F32, tag="ab2")
                nc.vector.tensor_mul(ab2[:, :, 0], gb2[:, :, 0], rs2[:])
                for w in range(NW):
                    nc.vector.tensor_mul(ab2[:, w, 1:2], ab2[:, w, 0:1],
                                         mv2_tiles[w][:, 0:1])
                nc.vector.tensor_sub(ab2[:, :, 1], gb2[:, :, 1], ab2[:, :, 1])

                # ---- Phase 3 ----
                with (
                    tc.tile_pool(name="p3", bufs=3) as p3,
                    tc.tile_pool(name="p3t", bufs=2, space="PSUM") as p3t,
                    tc.tile_pool(name="p3f1", bufs=2, space="PSUM") as p3f1,
                    tc.tile_pool(name="p3f2", bufs=1, space="PSUM") as p3f2,
                ):
                    for w in range(NW):
                        nw = _win_nodes(w)
                        x2w = x2_tiles[w]
                        h2 = p3.tile([128, T * D], BF16, tag="h2")
                        nc.scalar.activation(h2[:nw], x2w[:nw], AF.Identity,
                                             bias=ab2[:nw, w, 1:2],
                                             scale=ab2[:nw, w, 0:1])
                        tp = p3t.tile([128, 6, 128], BF16, tag="tpdd")
                        for t2 in range(6):
                            nc.tensor.transpose(
                                tp[:, t2, 0:nw],
                                h2[:nw, t2 * 128:(t2 + 1) * 128],
                                ident[:nw, :nw])
                        h2t = p3.tile([128, 6, 128], BF16, tag="h2t")
                        nc.vector.tensor_copy(h2t[:, :, 0:nw], tp[:, :, 0:nw])
                        if nw < 128:
                            nc.vector.memset(h2t[:, :, nw:128], 0.0)
                        fft = p3f2.tile([128, 6, 128], F32, tag="fft")
                        for par in range(2):
                            pb = par * 64
                            f1 = p3f1.tile([128, 768], F32, tag="f1")
                            rhs = h2t[pb:pb + 64, :, :].rearrange(
                                "p a b -> p (a b)")
                            nc.tensor.matmul(f1[:, 0:512], ffw1[pb:pb + 64, :],
                                             rhs[:, 0:512],
                                             start=True, stop=True)
                            nc.tensor.matmul(f1[:, 512:768], ffw1[pb:pb + 64, :],
                                             rhs[:, 512:768],
                                             start=True, stop=True)
                            g1 = p3.tile([128, 768], BF16, tag="g1")
                            nc.scalar.activation(g1[:], f1[:], AF.Gelu,
                                                 bias=ffb1[:])
                            fsl = fft[pb:pb + 64, :, :].rearrange(
                                "p a b -> p (a b)")
                            nc.tensor.matmul(fsl[:, 0:512], ffw2[:],
                                             g1[:, 0:512], start=True, stop=True)
                            nc.tensor.matmul(fsl[:, 512:768], ffw2[:],
                                             g1[:, 512:768],
                                             start=True, stop=True)
                        fsb = p3.tile([128, 6, 128], BF16, tag="fsb")
                        nc.vector.tensor_copy(fsb[:], fft[:])
                        dd = p3t.tile([128, 6, 128], BF16, tag="tpdd")
                        for t2 in range(6):
                            nc.tensor.transpose(dd[0:nw, t2, :],
                                                fsb[:, t2, 0:nw], ident[:, :])
                        # restore standard (t,h,k) order on the final add
                        ot = p3.tile([128, T * D], F32, tag="ot")
                        nc.vector.tensor_add(
                            ot[:nw].rearrange("p (t h k) -> p t h k",
                                              h=4, k=16),
                            dd[:nw].rearrange("p a b -> p (a b)")
                            .rearrange("p (t k h) -> p t h k", k=16, h=4),
                            x2w[:nw].rearrange("p (t k h) -> p t h k",
                                               k=16, h=4))
                        nc.sync.dma_start(out_d[w * WIN:w * WIN + nw], ot[:nw])

    nc.compile()
    return nc


_CACHE = {}
_TRACE = False
_LAST_EXEC_NS = None


def _host_prep(inputs):
    x = np.asarray(inputs["x"], np.float32)
    xf = np.ascontiguousarray(x.reshape(N, T * D))
    xh_full = xf.astype(BF16NP)
    B = 0
    for r in (1, 2):
        B = max(B, _max_blocks(np.asarray(inputs[f"src{r}"]),
                               np.asarray(inputs[f"dst{r}"])))

    bn1 = np.stack([np.asarray(inputs["bn1_g"], np.float32),
                    np.asarray(inputs["bn1_b"], np.float32)], axis=1)
    bn2 = np.stack([np.asarray(inputs["bn2_g"], np.float32),
                    np.asarray(inputs["bn2_b"], np.float32)], axis=1)
    # permute d -> (k, h) order in ffn weights to match the z table layout
    perm = (np.arange(64).reshape(H, DH).T.reshape(-1))  # pos i holds h*16+k
    ffw1 = np.asarray(inputs["ff_w1"], np.float32)[perm, :]
    ffw2 = np.asarray(inputs["ff_w2"], np.float32)[:, perm]
    common = {
        "ffw1": np.ascontiguousarray(ffw1),
        "ffb1": np.ascontiguousarray(
            np.asarray(inputs["ff_b1"], np.float32).reshape(DFF, 1)),
        "ffw2": np.ascontiguousarray(ffw2),
        "ffb2": np.ascontiguousarray(
            np.asarray(inputs["ff_b2"], np.float32).reshape(D, 1)),
        "ident": np.eye(128, dtype=BF16NP),
    }
    for r in (1, 2):
        W = np.asarray(inputs[f"W{r}"], np.float32)      # [D, H, DH]
        al = np.asarray(inputs[f"al{r}"], np.float32)    # [H, DH]
        ar = np.asarray(inputs[f"ar{r}"], np.float32)
        wal = np.einsum("dhk,hk->dh", W, al)
        war = np.einsum("dhk,hk->dh", W, ar)
        wkh = W.transpose(0, 2, 1).reshape(D, D)         # cols (k, h)
        common[f"Wx{r}"] = np.ascontiguousarray(
            np.concatenate([wkh, wal, war], axis=1))

    in_maps = []
    for m in range(NCORES):
        lo = m * CHUNK
        base = 0 if m % 2 == 0 else HALF
        im = dict(common)
        xh = np.zeros((HBLK * 128, T * D), BF16NP)
        xh[0:HALF] = xh_full[base:base + HALF]
        im["xh"] = xh
        b1r = np.zeros((HBLK * 128, 2), np.float32)
        b1r[:, 0] = 1.0
        b1r[0:HALF] = bn1[base:base + HALF]
        # device layout [128, HBLK, 2]: partition p holds row j*128+p
        im["bn1r"] = np.ascontiguousarray(
            b1r.reshape(HBLK, 128, 2).transpose(1, 0, 2).reshape(128, HBLK * 2))
        im["xc"] = np.ascontiguousarray(xf[lo:lo + CHUNK])
        im["bn2c"] = np.ascontiguousarray(bn2[lo:lo + CHUNK])
        ni = np.full(NW * 128, -1, np.int16)
        for w in range(NW):
            nw = _win_nodes(w)
            ni[w * 128:w * 128 + nw] = lo + w * WIN + np.arange(nw)
        im["nodeidx"] = _idx16(ni)
        for r in (1, 2):
            src16, S = _prep_core_rel(
                np.asarray(inputs[f"src{r}"]), np.asarray(inputs[f"dst{r}"]),
                lo, B)
            im[f"srcidx{r}"] = src16
            im[f"S{r}"] = S
        in_maps.append(im)
    return B, in_maps


def kernel(**inputs):
    B, in_maps = _host_prep(inputs)
    if B not in _CACHE:
        _CACHE[B] = _build_program(B)
    nc = _CACHE[B]
    global _LAST_EXEC_NS
    res = run_bass_kernel_spmd(nc, in_maps, core_ids=list(range(NCORES)),
                               trace=_TRACE)
    _LAST_EXEC_NS = res.exec_time_ns
    out = np.concatenate([res.results[m]["OUT"] for m in range(NCORES)], axis=0)
    return out.reshape(N, T, D).astype(np.float32)


# revision 39
# speedup vs baseline: 1.0028x; 1.0028x over previous
"""Trainium2 Bass kernel for nn_EncoderLayer (GNN message passing, 2-relation GAT).

v3 design (pair-shared z-table):
  - Each trn2 chip pair (2p, 2p+1) shares HBM; the projection table
    ZT[10000, 896] bf16 (per rel) lives in pair-Shared DRAM. The even core
    computes+writes rows [0:5000), the odd core rows [5000:10000) (If-branch
    on core parity picks write addresses; compute uses per-core input data).
    A tiny pair AllGather is the phase-1 -> phase-2 barrier.
  - Table row: 12 x [z(t) 64 (k,h-order) | el(t) 4] | er 48 | pad. The
    (k,h) column order (host-permuted weights) makes the phase-2 ex*z
    broadcast multiply a packed-inner-dim DVE op (2x mode) with no
    scalar broadcast-exp. er for the window's own dst rows is gathered
    back from the table tail (256B suffix gather, -1 padded idx).
  - BN rsqrt = DVE reciprocal + scalar Sqrt; BN2 coefs batched between
    phases => only ~4 activation-table loads total (sqrt/exp/sqrt/gelu).
  - er expansion to edges on PE (S-block transposes, batched PSUM->SBUF
    copy, one-hot matmul with er rows).
  - Phase 2: gather z rows per edge, ex = exp(leaky(el+er)),
    msg = [ex*z | ex], segment-sum via S-matmuls in PSUM, alpha-normalize,
    residual, BN2 stats. Phase 3: BN2 apply + FFN (t-pair transposes,
    host-permuted ffw1 rows / ffw2 cols; final add restores layout).
"""

import sys

sys.path.insert(0, "/opt/trn_rl_repo")

import numpy as np
import ml_dtypes

import concourse.bass as bass
import concourse.bacc as bacc
import concourse.tile as tile
import concourse.mybir as mybir
from concourse.bass_utils import run_bass_kernel_spmd

F32 = mybir.dt.float32
BF16 = mybir.dt.bfloat16
I16 = mybir.dt.int16
AF = mybir.ActivationFunctionType
ALU = mybir.AluOpType
BF16NP = ml_dtypes.bfloat16

N, T, D, H, DH, DFF = 10000, 12, 64, 4, 16, 128
NCORES = 8
CHUNK = N // NCORES          # 1250
WIN = 128
NW = (CHUNK + WIN - 1) // WIN  # 10
HALF = N // 2                # 5000 rows per pair member
HBLK = (HALF + 127) // 128   # 40 blocks (last = 8 rows)
ZROW = 896                   # bf16 elems per table row (1792 B)
EPS = 1e-5
NEG_SLOPE = 0.2
_SIM = False                 # True: branch-free variant for TimelineSim
_DEBUG = False               # True: add ZDBG output dumping table rows


def _win_nodes(w):
    return min(WIN, CHUNK - w * WIN)


def _hblk_nodes(j):
    return min(128, HALF - j * 128)


def _idx16(arr):
    """dma_gather index layout: idx i at [i%16, i//16], tiled to 128 rows."""
    return np.ascontiguousarray(np.tile(arr.reshape(-1, 16).T, (8, 1)))


def _prep_core_rel(src, dst, lo, B):
    hi = lo + CHUNK
    sel = (dst >= lo) & (dst < hi)
    es = src[sel].astype(np.int64)
    ed = (dst[sel] - lo).astype(np.int64)
    order = np.argsort(ed, kind="stable")
    es, ed = es[order], ed[order]
    L = NW * B * 128
    src_arr = np.zeros(L, np.int16)
    S = np.zeros((NW, 128, B * 128), BF16NP)
    wstart = np.searchsorted(ed, np.arange(NW) * WIN)
    wend = np.searchsorted(ed, np.arange(1, NW + 1) * WIN)
    for w in range(NW):
        seg_src = es[wstart[w]:wend[w]]
        seg_dst = ed[wstart[w]:wend[w]]
        cnt = len(seg_src)
        assert cnt <= B * 128
        base = w * B * 128
        src_arr[base:base + cnt] = seg_src
        i = np.arange(cnt)
        S[w, i % 128, (i // 128) * 128 + (seg_dst - w * WIN)] = 1.0
    return _idx16(src_arr), S


def _max_blocks(src, dst):
    best = 0
    for m in range(NCORES):
        lo = m * CHUNK
        sel = (dst >= lo) & (dst < lo + CHUNK)
        ed = dst[sel] - lo
        cnt = np.bincount(ed // WIN, minlength=NW)
        best = max(best, int(np.max((cnt + 127) // 128)))
    return best


def _build_program(B):
    nc = bacc.Bacc("TRN2", target_bir_lowering=False, debug=False,
                   num_devices=NCORES)
    BL = B * 128
    L16W = BL // 16

    # ---- DRAM tensors ----
    xh_in = nc.dram_tensor("xh", [HBLK * 128, T * D], BF16, kind="ExternalInput")
    bn1r_in = nc.dram_tensor("bn1r", [128, HBLK * 2], F32, kind="ExternalInput")
    xc_in = nc.dram_tensor("xc", [CHUNK, T * D], F32, kind="ExternalInput")
    bn2c_in = nc.dram_tensor("bn2c", [CHUNK, 2], F32, kind="ExternalInput")
    wx_in, s_in, si_in = [], [], []
    for r in (1, 2):
        wx_in.append(nc.dram_tensor(f"Wx{r}", [D, 72], F32, kind="ExternalInput"))
        s_in.append(nc.dram_tensor(f"S{r}", [NW, 128, BL], BF16,
                                   kind="ExternalInput"))
        si_in.append(nc.dram_tensor(f"srcidx{r}", [128, NW * L16W], I16,
                                    kind="ExternalInput"))
    ni_in = nc.dram_tensor("nodeidx", [128, NW * 8], I16, kind="ExternalInput")
    ffw1_in = nc.dram_tensor("ffw1", [D, DFF], F32, kind="ExternalInput")
    ffb1_in = nc.dram_tensor("ffb1", [DFF, 1], F32, kind="ExternalInput")
    ffw2_in = nc.dram_tensor("ffw2", [DFF, D], F32, kind="ExternalInput")
    ffb2_in = nc.dram_tensor("ffb2", [D, 1], F32, kind="ExternalInput")
    ident_in = nc.dram_tensor("ident", [128, 128], BF16, kind="ExternalInput")
    out_d = nc.dram_tensor("OUT", [CHUNK, T * D], F32, kind="ExternalOutput")
    if _DEBUG:
        zdbg_d = nc.dram_tensor("ZDBG", [2, 128, 1792], BF16,
                                kind="ExternalOutput")

    ZTall = nc.dram_tensor("ZT", [N, 2 * ZROW], BF16, kind="Internal",
                           addr_space="Shared")


    with tile.TileContext(nc) as tc:
        with tc.tile_pool(name="const", bufs=1) as cpool:
            # ---- Phase 0 ----
            ident = cpool.tile([128, 128], BF16)
            nc.sync.dma_start(ident[:], ident_in[:])
            wd2 = cpool.tile([128, 2, 2, 72], BF16, tag="wd2")
            nc.vector.memset(wd2[:], 0.0)
            for r in range(2):
                wf = cpool.tile([D, 72], F32, tag="wf", bufs=2)
                nc.sync.dma_start(wf[:], wx_in[r][:])
                for par in range(2):
                    nc.vector.tensor_copy(
                        wd2[par * 64:par * 64 + 64, r, par, :], wf[:])
            ffw1 = cpool.tile([128, DFF], BF16)
            t1 = cpool.tile([D, DFF], F32, tag="t1")
            nc.sync.dma_start(t1[:], ffw1_in[:])
            nc.vector.tensor_copy(ffw1[0:D, :], t1[:])
            nc.sync.dma_start(ffw1[64:128, :], ffw1[0:64, :])
            ffw2 = cpool.tile([DFF, D], BF16)
            t2 = cpool.tile([DFF, D], F32, tag="t2")
            nc.sync.dma_start(t2[:], ffw2_in[:])
            nc.vector.tensor_copy(ffw2[:], t2[:])
            ffb1 = cpool.tile([DFF, 1], F32)
            nc.sync.dma_start(ffb1[:], ffb1_in[:])
            ffb2r = cpool.tile([128, 1], F32)
            nc.sync.dma_start(ffb2r[0:64, :], ffb2_in[:])
            nc.sync.dma_start(ffb2r[64:128, :], ffb2_in[:])
            si_sb = []
            for r in range(2):
                sit = cpool.tile([128, NW * L16W], I16, tag=f"si{r}")
                nc.sync.dma_start(sit[:], si_in[r][:])
                si_sb.append(sit)
            ni_sb = cpool.tile([128, NW * 8], I16, tag="ni")
            nc.sync.dma_start(ni_sb[:], ni_in[:])
            gb1 = cpool.tile([128, HBLK, 2], F32, tag="gb1")
            nc.sync.dma_start(gb1[:].rearrange("p a b -> p (a b)"), bn1r_in[:])

            if not _SIM:
                par_reg = nc.sync.alloc_register("par")
                nc.sync.reg_mod(par_reg, nc.sync.partition_id(), 2)
                par_sv = nc.sync.snap(par_reg, min_val=0, max_val=1)

            # ---- Phase 1: 40 half-table blocks ----
            with (
                tc.tile_pool(name="p1", bufs=4) as p1,
                tc.tile_pool(name="p1s", bufs=3) as p1s,
                tc.tile_pool(name="p1t", bufs=2, space="PSUM") as p1t,
                tc.tile_pool(name="p1z", bufs=2, space="PSUM") as p1z,
                tc.tile_pool(name="zelp", bufs=4) as zelp,
            ):
                xt4 = None
                for j in range(HBLK):
                    n0 = j * 128
                    nb = _hblk_nodes(j)
                    if j % 4 == 0:
                        xt4 = p1.tile([128, 4, T * D], BF16, tag="xt")
                        nc.sync.dma_start(
                            xt4[:],
                            xh_in[j * 128:(j + 4) * 128].rearrange(
                                "(q p) c -> p q c", p=128))
                    xt = xt4[:, j % 4, :]
                    st6 = p1s.tile([128, 2, 6], F32, tag="st6")
                    nc.vector.bn_stats(st6[:nb, 0, :], xt[:nb, 0:384])
                    nc.vector.bn_stats(st6[:nb, 1, :], xt[:nb, 384:768])
                    mv = p1s.tile([128, 2], F32, tag="mv")
                    nc.vector.bn_aggr(mv[:nb], st6[:nb])
                    gb = gb1[:, j, :]
                    # rsqrt = sqrt(1/(v+eps)): no exp/ln tables needed
                    rs = p1s.tile([128, 1], F32, tag="rs")
                    nc.vector.tensor_scalar_add(rs[:nb], mv[:nb, 1:2], EPS)
                    nc.vector.reciprocal(rs[:nb], rs[:nb])
                    nc.scalar.activation(rs[:nb], rs[:nb], AF.Sqrt)
                    a = p1s.tile([128, 1], F32, tag="a")
                    nc.vector.tensor_mul(a[:nb], gb[:nb, 0:1], rs[:nb])
                    b = p1s.tile([128, 1], F32, tag="b")
                    nc.vector.tensor_mul(b[:nb], a[:nb], mv[:nb, 0:1])
                    nc.vector.tensor_sub(b[:nb], gb[:nb, 1:2], b[:nb])
                    h = p1.tile([128, T * D], BF16, tag="h")
                    nc.scalar.activation(h[:nb], xt[:nb], AF.Identity,
                                         bias=b[:nb], scale=a[:nb])
                    tp = p1t.tile([128, 6, 128], BF16, tag="tp")
                    for t2 in range(6):
                        nc.tensor.transpose(tp[:, t2, 0:nb],
                                            h[:nb, t2 * 128:(t2 + 1) * 128],
                                            ident[:nb, :nb])
                    ht = p1.tile([128, 6, 128], BF16, tag="ht")
                    nc.scalar.activation(ht[:, :, 0:nb], tp[:, :, 0:nb], AF.Copy)
                    zel = zelp.tile([128, 2, ZROW], BF16, tag="zel")
                    for r in range(2):
                        zp = p1z.tile([128, 6, 256], F32, tag="zp")
                        zp4 = zp[:, :, 0:144].rearrange(
                            "p a (q c) -> p a q c", c=72)
                        for t2 in range(6):
                            nc.tensor.matmul(
                                zp[:nb, t2, 0:144],
                                ht[:, t2, 0:nb],
                                wd2[:, r, :, :].rearrange("p a b -> p (a b)"),
                                start=True, stop=True)
                        for par in range(2):
                            # z+el fused: 68 cols per t, (k,h) weight order
                            outz = zel[0:nb, r, 0:816] \
                                .rearrange("p (a q c) -> p a q c",
                                           q=2, c=68)[:, :, par, :]
                            inz = zp4[0:nb, :, par, 0:68]
                            if par == 0:
                                nc.scalar.activation(outz, inz, AF.Copy)
                            else:
                                nc.vector.tensor_copy(outz, inz)
                        # er tail: cols 816:864, order (t, h)
                        oute = zel[0:nb, r, 816:864] \
                            .rearrange("p (a q c) -> p a q c", q=2, c=4)
                        ine = zp4[0:nb, :, :, 68:72]
                        nc.vector.tensor_copy(oute, ine)
                    if _SIM:
                        nc.sync.dma_start(ZTall[n0:n0 + nb, :],
                                          zel[0:nb, :, :])
                    else:
                        with tc.If(par_sv == 0):
                            nc.sync.dma_start(ZTall[n0:n0 + nb, :],
                                              zel[0:nb, :, :])
                        with tc.If(par_sv == 1):
                            nc.sync.dma_start(
                                ZTall[HALF + n0:HALF + n0 + nb, :],
                                zel[0:nb, :, :])

            tc.strict_bb_all_engine_barrier()

            # ---- pair barrier ----
            with tc.tile_pool(name="ccd", bufs=1, space="DRAM") as ccd:
                cbin = ccd.tile([1, 32], F32, tag="cbin")
                cbout = ccd.tile([8, 32], F32, tag="cbout")
                bar_sb = cpool.tile([1, 32], F32, tag="barsb")
                nc.vector.memset(bar_sb[:], 0.0)
                nc.gpsimd.dma_start(cbin[:], bar_sb[:])
                nc.gpsimd.collective_compute(
                    "AllGather", ALU.bypass,
                    replica_groups=[[0, 1, 2, 3, 4, 5, 6, 7]],
                    ins=[cbin[:].opt()], outs=[cbout[:].opt()],
                )
                nc.gpsimd.dma_start(bar_sb[:], cbout[0:1, :])
            tc.strict_bb_all_engine_barrier()

            if _DEBUG:
                zd = cpool.tile([128, 2, 1792], BF16, tag="zd")
                nc.gpsimd.dma_start(zd[:, 0, :], ZTall[0:128, :])
                nc.gpsimd.dma_start(zd[:, 1, :], ZTall[HALF:HALF + 128, :])
                nc.gpsimd.dma_start(zdbg_d[0], zd[:, 0, :])
                nc.gpsimd.dma_start(zdbg_d[1], zd[:, 1, :])

            # ---- Phase 2 ----
            x2_tiles = []
            mv2_tiles = []
            with (
                tc.tile_pool(name="x2p", bufs=NW) as x2p,
                tc.tile_pool(name="mv2p", bufs=NW) as mv2p,
                tc.tile_pool(name="abp", bufs=1) as abp,
            ):
                with (
                    tc.tile_pool(name="zg", bufs=2) as zgp,
                    tc.tile_pool(name="sp", bufs=2) as spp,
                    tc.tile_pool(name="erw", bufs=2) as erwp,
                    tc.tile_pool(name="lkp", bufs=2) as lkp,
                    tc.tile_pool(name="msg", bufs=6) as msgp,
                    tc.tile_pool(name="p2s", bufs=4) as p2s,
                    tc.tile_pool(name="p2t", bufs=2) as p2t,
                    tc.tile_pool(name="stsp", bufs=2) as stsp,
                    tc.tile_pool(name="msum", bufs=1, space="PSUM") as msump,
                    tc.tile_pool(name="erp", bufs=1, space="PSUM") as erpp,
                    tc.tile_pool(name="stp", bufs=1, space="PSUM") as stpp,
                ):
                    for w in range(NW):
                        nw = _win_nodes(w)
                        ms = msump.tile([128, 2, 1024], F32, tag="msum")
                        for r in range(2):
                            ssb = spp.tile([128, BL], BF16, tag=f"ssb{r}")
                            nc.sync.dma_start(ssb[:], s_in[r][w])
                            zg = zgp.tile([128, B, ZROW], BF16, tag=f"zg{r}")
                            nc.gpsimd.dma_gather(
                                zg[:], ZTall[:, r * ZROW:r * ZROW + ZROW],
                                si_sb[r][:, w * L16W:(w + 1) * L16W],
                                BL, BL, ZROW, elem_step=2 * ZROW,
                                single_packet=False)
                            erw = erwp.tile([128, 128], BF16, tag=f"erw{r}")
                            nc.vector.memset(erw[:], 0.0)
                            nc.gpsimd.dma_gather(
                                erw[:].unsqueeze(1),
                                ZTall[:, r * ZROW + 768:r * ZROW + 896],
                                ni_sb[:, w * 8:(w + 1) * 8],
                                128, 128, 128, elem_step=2 * ZROW,
                                single_packet=False)
                            # er expansion: St_b = S_b^T (batched), one-hot mm
                            stp = stpp.tile([128, B, 128], BF16, tag="stp")
                            for b in range(B):
                                nc.tensor.transpose(
                                    stp[:, b, :],
                                    ssb[:, b * 128:(b + 1) * 128], ident[:])
                            sts = stsp.tile([128, B, 128], BF16, tag="sts")
                            nc.vector.tensor_copy(sts[:], stp[:])
                            er_ps = erpp.tile([128, B, 64], F32, tag="erps")
                            for b in range(B):
                                nc.tensor.matmul(
                                    er_ps[:, b, 0:T * H], sts[:, b, :],
                                    erw[:, 48:96], start=True, stop=True)
                            lk = lkp.tile([128, B, T * H], BF16, tag="lk")
                            nc.vector.tensor_add(
                                lk[:].rearrange("p b (t h) -> p b t h", h=H),
                                zg[:, :, 0:816].rearrange(
                                    "p b (t c) -> p b t c", c=68)[:, :, :, 64:68],
                                er_ps[:, :, 0:T * H].rearrange(
                                    "p b (t h) -> p b t h", h=H))
                            nc.vector.scalar_tensor_tensor(
                                lk[:], lk[:], NEG_SLOPE, lk[:], ALU.mult, ALU.max)
                            for b in range(B):
                                msgb = msgp.tile([128, 816], BF16, tag="msg")
                                nc.scalar.activation(msgb[:, 768:816],
                                                     lk[:, b, :], AF.Exp)
                                zv = zg[:, b, 0:816].rearrange(
                                    "p (t c) -> p t c", c=68)[:, :, 0:64] \
                                    .rearrange("p t (k h) -> p t k h", h=4)
                                nc.vector.tensor_mul(
                                    msgb[:, 0:768].rearrange(
                                        "p (t k h) -> p t k h", k=16, h=4),
                                    zv,
                                    msgb[:, 768:816].rearrange(
                                        "p (t h) -> p t h", h=4).unsqueeze(2)
                                    .broadcast_to((128, T, 16, H)))
                                lhsT = ssb[:, b * 128:(b + 1) * 128]
                                nc.tensor.matmul(ms[:, r, 0:512], lhsT,
                                                 msgb[:, 0:512],
                                                 start=(b == 0),
                                                 stop=(b == B - 1))
                                nc.tensor.matmul(ms[:, r, 512:816], lhsT,
                                                 msgb[:, 512:816],
                                                 start=(b == 0),
                                                 stop=(b == B - 1))
                        # epilogue
                        xcw = p2t.tile([128, T * D], F32, tag="xcw")
                        nc.sync.dma_start(xcw[:nw], xc_in[w * WIN:w * WIN + nw])
                        x2w = x2p.tile([128, T * D], F32, tag="x2")
                        mtmp = p2t.tile([128, T * D], F32, tag="mtmp")
                        for r in range(2):
                            rec = p2s.tile([128, T * H], F32, tag="rec")
                            nc.vector.tensor_scalar_max(
                                rec[:nw], ms[:nw, r, 768:816], 1e-16)
                            nc.vector.reciprocal(rec[:nw], rec[:nw])
                            rb = rec[:nw].rearrange(
                                "p (t h) -> p t h", h=H).unsqueeze(2) \
                                .broadcast_to((nw, T, 16, H))
                            dst = (x2w if r == 0 else mtmp)
                            nc.vector.tensor_mul(
                                dst[:nw].rearrange(
                                    "p (t k h) -> p t k h", k=16, h=4),
                                ms[:nw, r, 0:768].rearrange(
                                    "p (t k h) -> p t k h", k=16, h=4), rb)
                        # x2 kept in (t,k,h) order; xc viewed permuted
                        nc.vector.tensor_add(x2w[:nw], x2w[:nw], mtmp[:nw])
                        nc.vector.tensor_add(
                            x2w[:nw].rearrange("p (t k h) -> p t k h",
                                               k=16, h=4),
                            x2w[:nw].rearrange("p (t k h) -> p t k h",
                                               k=16, h=4),
                            xcw[:nw].rearrange("p (t h k) -> p t k h",
                                               h=4, k=16))
                        x2_tiles.append(x2w)
                        st6 = p2s.tile([128, 2, 6], F32, tag="st6b")
                        nc.vector.bn_stats(st6[:nw, 0, :], x2w[:nw, 0:384])
                        nc.vector.bn_stats(st6[:nw, 1, :], x2w[:nw, 384:768])
                        mv2 = mv2p.tile([128, 2], F32, tag="mvb")
                        nc.vector.bn_aggr(mv2[:nw], st6[:nw])
                        mv2_tiles.append(mv2)

                # ---- batched BN2 coefs (one Sqrt op: no table thrash) ----
                gb2 = abp.tile([128, NW, 2], F32, tag="gb2")
                for w in range(NW):
                    nw = _win_nodes(w)
                    nc.sync.dma_start(gb2[:nw, w, :],
                                      bn2c_in[w * WIN:w * WIN + nw])
                rs2 = abp.tile([128, NW], F32, tag="rs2")
                for w in range(NW):
                    nc.vector.tensor_scalar_add(rs2[:, w:w + 1],
                                                mv2_tiles[w][:, 1:2], EPS)
                nc.vector.reciprocal(rs2[:], rs2[:])
                nc.scalar.activation(rs2[:], rs2[:], AF.Sqrt)
                ab2 = abp.tile([128, NW, 2], F32, tag="ab2")
                nc.vector.tensor_mul(ab2[:, :, 0], gb2[:, :, 0], rs2[:])
                for w in range(NW):
                    nc.vector.tensor_mul(ab2[:, w, 1:2], ab2[:, w, 0:1],
                                         mv2_tiles[w][:, 0:1])
                nc.vector.tensor_sub(ab2[:, :, 1], gb2[:, :, 1], ab2[:, :, 1])

                # ---- Phase 3 ----
                with (
                    tc.tile_pool(name="p3", bufs=3) as p3,
                    tc.tile_pool(name="p3t", bufs=2, space="PSUM") as p3t,
                    tc.tile_pool(name="p3f1", bufs=2, space="PSUM") as p3f1,
                    tc.tile_pool(name="p3f2", bufs=1, space="PSUM") as p3f2,
                ):
                    for w in range(NW):
                        nw = _win_nodes(w)
                        x2w = x2_tiles[w]
                        h2 = p3.tile([128, T * D], BF16, tag="h2")
                        nc.scalar.activation(h2[:nw], x2w[:nw], AF.Identity,
                                             bias=ab2[:nw, w, 1:2],
                                             scale=ab2[:nw, w, 0:1])
                        tp = p3t.tile([128, 6, 128], BF16, tag="tpdd")
                        for t2 in range(6):
                            nc.tensor.transpose(
                                tp[:, t2, 0:nw],
                                h2[:nw, t2 * 128:(t2 + 1) * 128],
                                ident[:nw, :nw])
                        h2t = p3.tile([128, 6, 128], BF16, tag="h2t")
                        nc.scalar.activation(h2t[:, :, 0:nw], tp[:, :, 0:nw],
                                             AF.Copy)
                        if nw < 128:
                            nc.vector.memset(h2t[:, :, nw:128], 0.0)
                        fft = p3f2.tile([128, 6, 128], F32, tag="fft")
                        for par in range(2):
                            pb = par * 64
                            f1 = p3f1.tile([128, 768], F32, tag="f1")
                            rhs = h2t[pb:pb + 64, :, :].rearrange(
                                "p a b -> p (a b)")
                            nc.tensor.matmul(f1[:, 0:512], ffw1[pb:pb + 64, :],
                                             rhs[:, 0:512],
                                             start=True, stop=True)
                            nc.tensor.matmul(f1[:, 512:768], ffw1[pb:pb + 64, :],
                                             rhs[:, 512:768],
                                             start=True, stop=True)
                            g1 = p3.tile([128, 768], BF16, tag="g1")
                            nc.scalar.activation(g1[:], f1[:], AF.Gelu,
                                                 bias=ffb1[:])
                            fsl = fft[pb:pb + 64, :, :].rearrange(
                                "p a b -> p (a b)")
                            nc.tensor.matmul(fsl[:, 0:512], ffw2[:],
                                             g1[:, 0:512], start=True, stop=True)
                            nc.tensor.matmul(fsl[:, 512:768], ffw2[:],
                                             g1[:, 512:768],
                                             start=True, stop=True)
                        fsb = p3.tile([128, 6, 128], BF16, tag="fsb")
                        nc.scalar.activation(fsb[:], fft[:], AF.Copy)
                        dd = p3t.tile([128, 6, 128], BF16, tag="tpdd")
                        for t2 in range(6):
                            nc.tensor.transpose(dd[0:nw, t2, :],
                                                fsb[:, t2, 0:nw], ident[:, :])
                        # restore standard (t,h,k) order on the final add
                        ot = p3.tile([128, T * D], F32, tag="ot")
                        nc.vector.tensor_add(
                            ot[:nw].rearrange("p (t h k) -> p t h k",
                                              h=4, k=16),
                            dd[:nw].rearrange("p a b -> p (a b)")
                            .rearrange("p (t k h) -> p t h k", k=16, h=4),
                            x2w[:nw].rearrange("p (t k h) -> p t h k",
                                               k=16, h=4))
                        nc.sync.dma_start(out_d[w * WIN:w * WIN + nw], ot[:nw])

    nc.compile()
    return nc


_CACHE = {}
_TRACE = False
_LAST_EXEC_NS = None


def _host_prep(inputs):
    x = np.asarray(inputs["x"], np.float32)
    xf = np.ascontiguousarray(x.reshape(N, T * D))
    xh_full = xf.astype(BF16NP)
    B = 0
    for r in (1, 2):
        B = max(B, _max_blocks(np.asarray(inputs[f"src{r}"]),
                               np.asarray(inputs[f"dst{r}"])))

    bn1 = np.stack([np.asarray(inputs["bn1_g"], np.float32),
                    np.asarray(inputs["bn1_b"], np.float32)], axis=1)
    bn2 = np.stack([np.asarray(inputs["bn2_g"], np.float32),
                    np.asarray(inputs["bn2_b"], np.float32)], axis=1)
    # permute d -> (k, h) order in ffn weights to match the z table layout
    perm = (np.arange(64).reshape(H, DH).T.reshape(-1))  # pos i holds h*16+k
    ffw1 = np.asarray(inputs["ff_w1"], np.float32)[perm, :]
    ffw2 = np.asarray(inputs["ff_w2"], np.float32)[:, perm]
    common = {
        "ffw1": np.ascontiguousarray(ffw1),
        "ffb1": np.ascontiguousarray(
            np.asarray(inputs["ff_b1"], np.float32).reshape(DFF, 1)),
        "ffw2": np.ascontiguousarray(ffw2),
        "ffb2": np.ascontiguousarray(
            np.asarray(inputs["ff_b2"], np.float32).reshape(D, 1)),
        "ident": np.eye(128, dtype=BF16NP),
    }
    for r in (1, 2):
        W = np.asarray(inputs[f"W{r}"], np.float32)      # [D, H, DH]
        al = np.asarray(inputs[f"al{r}"], np.float32)    # [H, DH]
        ar = np.asarray(inputs[f"ar{r}"], np.float32)
        wal = np.einsum("dhk,hk->dh", W, al)
        war = np.einsum("dhk,hk->dh", W, ar)
        wkh = W.transpose(0, 2, 1).reshape(D, D)         # cols (k, h)
        common[f"Wx{r}"] = np.ascontiguousarray(
            np.concatenate([wkh, wal, war], axis=1))

    in_maps = []
    for m in range(NCORES):
        lo = m * CHUNK
        base = 0 if m % 2 == 0 else HALF
        im = dict(common)
        xh = np.zeros((HBLK * 128, T * D), BF16NP)
        xh[0:HALF] = xh_full[base:base + HALF]
        im["xh"] = xh
        b1r = np.zeros((HBLK * 128, 2), np.float32)
        b1r[:, 0] = 1.0
        b1r[0:HALF] = bn1[base:base + HALF]
        # device layout [128, HBLK, 2]: partition p holds row j*128+p
        im["bn1r"] = np.ascontiguousarray(
            b1r.reshape(HBLK, 128, 2).transpose(1, 0, 2).reshape(128, HBLK * 2))
        im["xc"] = np.ascontiguousarray(xf[lo:lo + CHUNK])
        im["bn2c"] = np.ascontiguousarray(bn2[lo:lo + CHUNK])
        ni = np.full(NW * 128, -1, np.int16)
        for w in range(NW):
            nw = _win_nodes(w)
            ni[w * 128:w * 128 + nw] = lo + w * WIN + np.arange(nw)
        im["nodeidx"] = _idx16(ni)
        for r in (1, 2):
            src16, S = _prep_core_rel(
                np.asarray(inputs[f"src{r}"]), np.asarray(inputs[f"dst{r}"]),
                lo, B)
            im[f"srcidx{r}"] = src16
            im[f"S{r}"] = S
        in_maps.append(im)
    return B, in_maps


def kernel(**inputs):
    B, in_maps = _host_prep(inputs)
    if B not in _CACHE:
        _CACHE[B] = _build_program(B)
    nc = _CACHE[B]
    global _LAST_EXEC_NS
    res = run_bass_kernel_spmd(nc, in_maps, core_ids=list(range(NCORES)),
                               trace=_TRACE)
    _LAST_EXEC_NS = res.exec_time_ns
    out = np.concatenate([res.results[m]["OUT"] for m in range(NCORES)], axis=0)
    return out.reshape(N, T, D).astype(np.float32)
